# revision 54
# baseline (speedup 1.0000x reference)
# Trainium2 Bass kernel for nn_LogitsNew (dense_mlp).
#
#   u = gelu(x @ W_proj + b_proj)                       [B, D]
#   logits = (u @ W_u)[:, None, :] + ee @ W_e           [B, N, C]
#
# Sharding: data-parallel over batch B across 8 cores (4 batches/core).
#
# Final design (~59-60us HW; f32r baseline was 95-102us):
#  - All matmul operands fp16 (PE rate matches f32r at 1 cyc/row but DMA
#    bytes halve); ee/x pre-transposed on host; fp16 output upcast on host.
#  - DMA queues drain in trigger order at full bandwidth, so trigger order
#    IS the delivery schedule: (eet_k, we_k) pairs first (k0 split in
#    halves for an early first matmul), then all of wp, then all of wu.
#    eet rides the sync ring; we+wp+wu ride gpsimd; scalar only loads x/b
#    and then is a pure compute engine (no descriptor backpressure).
#  - PE order: A (m-tiles 0-3, k-outer across all 8 PSUM banks - 8 matmuls
#    per k-round matches the pair arrival cadence, so the PE never gaps
#    and holds max p-state) | z | uT | y | m4..m7. z runs when wp is
#    resident, y when wu is resident - no PE stalls on DMA after the ramp.
#  - Every m-tile gets y added on the DVE (no rank-1 matmuls); all DVE ops
#    are emitted in firing order so the in-order engine never head-of-line
#    blocks. m-tiles 0-4 drain (ACT/DVE halves in parallel) then add+store
#    per half; m-tiles 5-7 skip the drain: their PSUM banks are never
#    reused, so the DVE reads the f32 PSUM half directly, adds the fp16
#    y-broadcast, writes the fp16 out tile, and each half stores at once.
#  - GELU activation table preloaded at boot; y broadcast per batch on
#    gpsimd in batch order so every add's operand lands before it fires.

import sys

if "/opt/trn_rl_repo" not in sys.path:
    sys.path.insert(0, "/opt/trn_rl_repo")

import numpy as np

import concourse.bass as bass
import concourse.mybir as mybir
import concourse.tile as tile
from concourse import bacc
from concourse.bass_utils import run_bass_kernel_spmd
from concourse.masks import make_identity

P = 128
B, N, D, C = 32, 256, 1024, 1024
NCORES = 8
BPC = B // NCORES          # batches per core
KT = D // P                # 8 k-tiles over the contraction dim
FD = 512                   # matmul moving free dim (one PSUM bank of fp32)
NT = N // P                # 2 n-tiles per batch
MT = BPC * NT              # 8 m-tiles per core
BN = BPC * N               # 1024 columns of eeT per core

F32 = mybir.dt.float32
F16 = mybir.dt.float16
F32R = mybir.dt.float32r
GELU = mybir.ActivationFunctionType.Gelu

_CACHE = {}


def _build():
    if "nc" in _CACHE:
        return _CACHE["nc"]

    nc = bacc.Bacc("TRN2", target_bir_lowering=False, debug=False, num_devices=NCORES)

    # Host-prepped fp16 inputs (see run()).
    eet = nc.dram_tensor("ee_t", [KT, P, BN], F16, kind="ExternalInput").ap()
    we = nc.dram_tensor("w_e", [KT, P, C], F16, kind="ExternalInput").ap()
    wu = nc.dram_tensor("w_u", [KT, P, C], F16, kind="ExternalInput").ap()
    wp = nc.dram_tensor("w_p", [KT, P, C], F16, kind="ExternalInput").ap()
    xt = nc.dram_tensor("x_t", [KT, P, BPC], F16, kind="ExternalInput").ap()
    bp = nc.dram_tensor("b_p", [1, D], F16, kind="ExternalInput").ap()
    out = nc.dram_tensor("logits", [BPC, N, C], F16, kind="ExternalOutput").ap()

    eet3 = eet.rearrange("k p n -> p k n")
    we3 = we.rearrange("k p c -> p k c")
    wu3 = wu.rearrange("k p c -> p k c")
    wp3 = wp.rearrange("k p c -> p k c")
    xt3 = xt.rearrange("k p b -> p k b")

    with tile.TileContext(nc) as tc:
        with (
            tc.tile_pool(name="const", bufs=1) as cpool,
            tc.tile_pool(name="weights", bufs=1) as wpool,
            tc.tile_pool(name="outs", bufs=1) as outpool,
            tc.tile_pool(name="mm_ps", bufs=8, space="PSUM") as mm_ps,
        ):
            eet_sb = wpool.tile([P, KT, BN], F16)
            we_sb = wpool.tile([P, KT, C], F16)
            wp_sb = wpool.tile([P, KT, C], F16)
            wu_sb = wpool.tile([P, KT, C], F16)
            xt_sb = cpool.tile([P, KT, BPC], F16)
            b_sb = cpool.tile([1, D], F16)

            # gpsimd-made constants, before its DMA triggers
            ident_f = cpool.tile([P, P], F32)
            make_identity(nc, ident_f)
            ones_f = cpool.tile([1, P], F32)
            nc.gpsimd.memset(ones_f, 1.0)

            # ---- DMA triggers: delivery order == trigger order ----
            nc.scalar.dma_start(xt_sb, xt3)
            nc.scalar.dma_start(b_sb, bp)
            HBN = BN // 2
            nc.sync.dma_start(eet_sb[:, 0, :HBN], eet3[:, 0, :HBN])
            nc.gpsimd.dma_start(we_sb[:, 0, :FD], we3[:, 0, :FD])
            nc.sync.dma_start(eet_sb[:, 0, HBN:], eet3[:, 0, HBN:])
            nc.gpsimd.dma_start(we_sb[:, 0, FD:], we3[:, 0, FD:])
            for k in range(1, KT):
                nc.sync.dma_start(eet_sb[:, k, :], eet3[:, k, :])
                nc.gpsimd.dma_start(we_sb[:, k, :], we3[:, k, :])
            for j in range(4):
                nc.gpsimd.dma_start(wp_sb[:, 2 * j : 2 * j + 2, :], wp3[:, 2 * j : 2 * j + 2, :])
            for j in range(4):
                nc.gpsimd.dma_start(wu_sb[:, 2 * j : 2 * j + 2, :], wu3[:, 2 * j : 2 * j + 2, :])

            # scalar-side constants + GELU table preload
            identr = cpool.tile([P, P], F32R)
            nc.scalar.copy(identr, ident_f)
            ones16 = cpool.tile([1, P], F16)
            nc.scalar.copy(ones16, ones_f)
            junk = cpool.tile([1, P], F32R)
            nc.scalar.activation(junk, ones_f, GELU)

            out_tiles = {}
            for mt in range(MT):
                out_tiles[mt] = outpool.tile([P, C], F16, tag=f"o{mt}", name=f"o_{mt}")

            u16 = cpool.tile([BPC, C], F32R)
            uT = cpool.tile([P, KT, BPC], F16)
            y_sb = cpool.tile([BPC, C], F16)
            y_row = cpool.tile([1, BPC, C], F16)
            ybc0 = cpool.tile([P, C], F16)
            ybc1 = cpool.tile([P, C], F16)

            def mt_psum(mt):
                return [
                    mm_ps.tile([P, FD], F32, tag="mm", name=f"mm_{mt}_{ch}")
                    for ch in range(2)
                ]

            def mt_matmuls(mt, ps, rank1):
                b, nh = divmod(mt, NT)
                col = b * N + nh * P
                for k in range(KT):
                    lhsT = eet_sb[:, k, col : col + P]
                    for ch in range(2):
                        nc.tensor.matmul(
                            ps[ch],
                            lhsT,
                            we_sb[:, k, ch * FD : (ch + 1) * FD],
                            start=(k == 0),
                            stop=(k == KT - 1) and not rank1,
                        )
                if rank1:
                    for ch in range(2):
                        nc.tensor.matmul(
                            ps[ch],
                            ones16[:1, :P],
                            y_row[:1, b, ch * FD : (ch + 1) * FD],
                            start=False,
                            stop=True,
                        )

            def drain(mt, ps):
                # halves drain in parallel on ACT and DVE
                nc.scalar.copy(out_tiles[mt][:, :FD], ps[0])
                nc.vector.tensor_copy(out_tiles[mt][:, FD:], ps[1])

            def store(mt, eng):
                b, nh = divmod(mt, NT)
                eng.dma_start(out[b, nh * P : (nh + 1) * P, :], out_tiles[mt])

            # ---- group A: m-tiles 0-3, k-outer across all 8 banks ----
            # 8 matmuls per k-round matches the (eet_k, we_k) pair arrival
            # cadence, so the PE never gaps (and never drops p-state).
            psA = {mt: mt_psum(mt) for mt in (0, 1, 2, 3)}
            for k in range(KT):
                for mt in (0, 1, 2, 3):
                    b, nh = divmod(mt, NT)
                    col = b * N + nh * P
                    lhsT = eet_sb[:, k, col : col + P]
                    for ch in range(2):
                        nc.tensor.matmul(
                            psA[mt][ch],
                            lhsT,
                            we_sb[:, k, ch * FD : (ch + 1) * FD],
                            start=(k == 0),
                            stop=(k == KT - 1),
                        )
            for mt in (0, 1, 2, 3):
                drain(mt, psA[mt])

            # ---- z = x@Wp + b, gelu (wp is resident by now) ----
            for ch in range(2):
                cs = slice(ch * FD, (ch + 1) * FD)
                zp = mm_ps.tile([P, FD], F32, tag="mm", name=f"z_{ch}")
                for k in range(KT):
                    nc.tensor.matmul(
                        zp[:BPC], xt_sb[:, k, :], wp_sb[:, k, cs],
                        start=(k == 0), stop=False,
                    )
                nc.tensor.matmul(
                    zp[:BPC], ones16[:1, :BPC], b_sb[:1, cs],
                    start=False, stop=True,
                )
                nc.scalar.activation(u16[:, cs], zp[:BPC], GELU)

            # ---- uT, y = u@Wu ----
            for k in range(KT):
                tp = mm_ps.tile([P, BPC], F32R, tag="mm", name=f"tp_{k}")
                nc.tensor.transpose(
                    tp, u16[:BPC, k * P : (k + 1) * P], identr[:BPC, :BPC]
                )
                nc.scalar.copy(uT[:, k, :], tp)
            psy = {}
            for ch in range(2):
                cs = slice(ch * FD, (ch + 1) * FD)
                yp = mm_ps.tile([P, FD], F32, tag="mm", name=f"y_{ch}")
                for k in range(KT):
                    nc.tensor.matmul(
                        yp[:BPC], uT[:, k, :], wu_sb[:, k, cs],
                        start=(k == 0), stop=(k == KT - 1),
                    )
                psy[ch] = yp
            for ch in range(2):
                nc.scalar.copy(y_sb[:, ch * FD : (ch + 1) * FD], psy[ch][:BPC])
            nc.scalar.dma_start(y_row, y_sb)
            ybc2 = cpool.tile([P, C], F16)
            ybc3 = cpool.tile([P, C], F16)
            ybcs = [ybc0, ybc1, ybc2, ybc3]
            for b2 in range(BPC):
                nc.gpsimd.partition_broadcast(ybcs[b2], y_row[:1, b2, :])

            # No rank-1: every m-tile gets y added on the DVE after its
            # drain. DVE instructions are emitted in firing order so the
            # in-order engine never head-of-line blocks.
            # ---- m-tile 4 ----
            ps4 = mt_psum(4)
            mt_matmuls(4, ps4, rank1=False)
            drain(4, ps4)

            def add_store_split(mt):
                # per-half add->store so the tail never waits a full row
                b, nh = divmod(mt, NT)
                ns = slice(nh * P, (nh + 1) * P)
                o = out_tiles[mt]
                nc.vector.tensor_add(o[:, :FD], o[:, :FD], ybcs[b][:, :FD])
                nc.sync.dma_start(out[b, ns, :FD], o[:, :FD])
                nc.vector.tensor_add(o[:, FD:], o[:, FD:], ybcs[b][:, FD:])
                nc.gpsimd.dma_start(out[b, ns, FD:], o[:, FD:])

            # ---- adds for m-tiles 0-3 (fire as ybc0/ybc1 land) ----
            for mt in (0, 1, 2, 3):
                b, nh = divmod(mt, NT)
                o = out_tiles[mt]
                nc.vector.tensor_add(o, o, ybcs[b])
                store(mt, nc.gpsimd)
            add_store_split(4)

            # ---- m-tiles 5-7: fused PSUM-read adds (no drain step) ----
            # Their banks are never reused, so the DVE reads the f32 PSUM
            # half directly, adds the fp16 broadcast, writes the fp16 out
            # tile, and each half stores immediately.
            for mt in (5, 6, 7):
                b, nh = divmod(mt, NT)
                ns = slice(nh * P, (nh + 1) * P)
                ps = mt_psum(mt)
                mt_matmuls(mt, ps, rank1=False)
                o = out_tiles[mt]
                nc.vector.tensor_add(o[:, :FD], ps[0], ybcs[b][:, :FD])
                nc.sync.dma_start(out[b, ns, :FD], o[:, :FD])
                nc.vector.tensor_add(o[:, FD:], ps[1], ybcs[b][:, FD:])
                nc.gpsimd.dma_start(out[b, ns, FD:], o[:, FD:])

    nc.compile()
    _CACHE["nc"] = nc
    return nc


def run(inputs, trace=False, **kwargs):
    nc = _build()
    x = np.asarray(inputs["encoded_utterance"], np.float32)
    ee = np.asarray(inputs["element_embeddings"], np.float32)
    w = np.asarray(inputs["weight_matrix"], np.float32)
    wp = np.asarray(inputs["W_proj"], np.float32)
    bp = np.asarray(inputs["b_proj"], np.float32).reshape(1, D)

    # eeT per core: [b, n, (k p)] -> [k, p, (b n)]
    eet = np.ascontiguousarray(
        ee.reshape(NCORES, BPC, N, KT, P).transpose(0, 3, 4, 1, 2).reshape(NCORES, KT, P, BN)
    ).astype(np.float16)
    we = np.ascontiguousarray(w[D:].reshape(KT, P, C)).astype(np.float16)
    wu = np.ascontiguousarray(w[:D].reshape(KT, P, C)).astype(np.float16)
    wpr = np.ascontiguousarray(wp.reshape(KT, P, C)).astype(np.float16)
    # xT per core: [b, (k p)] -> [k, p, b]
    xtt = np.ascontiguousarray(
        x.reshape(NCORES, BPC, KT, P).transpose(0, 2, 3, 1)
    ).astype(np.float16)
    bp16 = bp.astype(np.float16)

    in_maps = []
    for i in range(NCORES):
        in_maps.append(
            {
                "ee_t": eet[i],
                "w_e": we,
                "w_u": wu,
                "w_p": wpr,
                "x_t": xtt[i],
                "b_p": bp16,
            }
        )

    res = run_bass_kernel_spmd(
        nc, in_maps, core_ids=list(range(NCORES)), trace=trace, **kwargs
    )
    full = np.concatenate(
        [r["logits"].astype(np.float32) for r in res.results], axis=0
    )
    return full, res


def kernel(**inputs) -> np.ndarray:
    return run(inputs, trace=False)[0]
